# revision 8
# baseline (speedup 1.0000x reference)
"""Trainium2 Bass kernel for nn_Conv3DNorm (modulated conv3d + demod + lrelu + clamp).

Reference math (styles == ones):
    dcoef[cout] = rsqrt(sum_{cin,kd,kh,kw} weight^2 + 1e-8)
    y = conv3d(x, weight * dcoef, pad=1)            # per-sample, stride 1
    y = leaky_relu(y + bias, 0.2) * sqrt(2)
    y = clip(y, -256, 256)

Sharding: data-parallel over batch. Core i processes sample i (B=8 == n_cores).

Design (v3):
  - dcoef folded into the weights on the host (it only depends on `weight`),
    so the device runs a plain conv + lrelu + clamp.
  - conv = 27 accumulated bf16 matmuls per 512-position output chunk over a
    zero-padded (H,W) input volume in SBUF. bf16 halves the PE weight-load
    (LDWEIGHTS) time vs f32r so it hides completely under the 512-row matmul
    stream (issue interval ~219 ns vs ~255 ns for f32r).
  - input path: each x depth-slice DMAs contiguously (128 x 2KB descriptors)
    into a staging ring on the sync HWDGE queue; a DVE copy scatters it into
    the padded layout right after a cheap contiguous full-plane memset (the
    old strided column memsets cost ~4.4 us each on DVE and delayed the
    first matmul).
  - weights are split into 3 tap-range DMAs with the first-needed taps
    (9..17, the kd=1 block used by the d=0 chunks) landing before the x
    slices on the sync queue; the rest follow. bias rides the Act queue.
  - y goes out over the Activation-engine HWDGE queue; the SWDGE path is
    completely unused, avoiding its ~6.5 us drain at teardown.
  - epilogue per chunk (exact lrelu, no act-table dependency):
        u  = psum*sqrt2 + sqrt2*bias      (Identity activation, Act engine)
        o  = max(alpha*u, u)              (DVE scalar_tensor_tensor)
        oc = clip(o, +-256) -> bf16       (DVE tensor_scalar)
    using lrelu(z) = max(z, alpha*z) for 0 < alpha < 1.
  - last chunk split into two 256-wide halves so its epilogue/DMA overlap
    the final matmuls.
"""

import os
import sys

for _p in (
    "/root/.axon_site",
    "/root/.axon_site/_ro/trn_rl_repo",
    "/root/.axon_site/_ro/pypackages",
):
    if os.path.isdir(_p) and _p not in sys.path:
        sys.path.insert(0, _p)

import numpy as np

import concourse.bass as bass  # noqa: F401
import concourse.mybir as mybir
import concourse.tile as tile
from concourse import bacc
from concourse.bass_utils import run_bass_kernel_spmd

# Problem constants (hardcoded per contract).
B = 8
CIN = 128
COUT = 128
D = H = W = 32
K = 3
NTAPS = K * K * K  # 27
HP = H + 2  # 34
WP = W + 2  # 34
NCHUNK = 64  # output chunks of 512 spatial positions: (d, half-of-H)
EPS = 1e-8
S1 = float(np.sqrt(2.0))  # ACT_GAIN * GAIN
CLAMP = 256.0
ALPHA = 0.2

LAST_RESULTS = None  # BassKernelResults of the most recent run (for test.py)

_CACHED = {}


def _valid_taps(d):
    return [t for t in range(NTAPS) if 0 <= d + t // 9 - 1 < D]


def _build_nc():
    dt = mybir.dt
    io_dt = dt.bfloat16

    nc = bacc.Bacc("TRN2")
    x_d = nc.dram_tensor("x", [CIN, D, H, W], io_dt, kind="ExternalInput")
    w_d = nc.dram_tensor("w", [CIN, NTAPS, COUT], io_dt, kind="ExternalInput")
    b_d = nc.dram_tensor("bias", [COUT, 1], dt.float32, kind="ExternalInput")
    y_d = nc.dram_tensor("y", [COUT, NCHUNK, 512], io_dt, kind="ExternalOutput")

    with tile.TileContext(nc) as tc:
        with (
            tc.tile_pool(name="big", bufs=1) as big,
            tc.tile_pool(name="small", bufs=1) as small,
            tc.tile_pool(name="stg", bufs=4) as stp,
            tc.tile_pool(name="epiv", bufs=4) as vp,
            tc.tile_pool(name="epio", bufs=4) as op,
        ):
            w_sb = big.tile([CIN, NTAPS, COUT], io_dt)
            bias_sb = small.tile([COUT, 1], dt.float32)
            xpad = big.tile([CIN, D, HP, WP], io_dt)

            def load_slice(d):
                # contiguous plane memset (halo zeroing) + contiguous DMA
                # (128 x 2KB descriptors) + DVE scatter into padded layout
                nc.vector.memset(xpad[:, d, :, :], 0.0)
                st = stp.tile([CIN, H, W], io_dt, name=f"st_{d}", tag="st")
                nc.sync.dma_start(st[:], x_d[:, d, :, :])
                nc.vector.tensor_scalar_add(
                    xpad[:, d, 1 : HP - 1, 1 : WP - 1], st[:], 0.0
                )

            # sync-queue order tuned so the first matmul's inputs land first:
            # slice 0, tap 9's weights (the first LDWEIGHTS), slice 1, then
            # the remaining weight taps in consumption order
            load_slice(0)
            nc.sync.dma_start(w_sb[:, 9:10, :], w_d[:, 9:10, :])
            nc.sync.dma_start(w_sb[:, 10:18, :], w_d[:, 10:18, :])
            load_slice(1)
            nc.sync.dma_start(w_sb[:, 18:27, :], w_d[:, 18:27, :])
            nc.sync.dma_start(w_sb[:, 0:9, :], w_d[:, 0:9, :])
            nc.scalar.dma_start(bias_sb[:], b_d[:])

            def epilogue(ps_ap, oc_ap, width, c, half=""):
                # u = sqrt2*psum + sqrt2*bias; o = max(alpha*u, u) = sqrt2*lrelu
                u = vp.tile([COUT, width], dt.float32)
                nc.scalar.activation(
                    u[:],
                    ps_ap,
                    mybir.ActivationFunctionType.Identity,
                    bias=bias_sb[:],
                    scale=S1,
                )
                o = vp.tile([COUT, width], dt.float32, name=f"o_{c}{half}", tag="o")
                nc.vector.scalar_tensor_tensor(
                    out=o[:],
                    in0=u[:],
                    scalar=ALPHA,
                    in1=u[:],
                    op0=mybir.AluOpType.mult,
                    op1=mybir.AluOpType.max,
                )
                nc.vector.tensor_scalar(
                    out=oc_ap,
                    in0=o[:],
                    scalar1=-CLAMP,
                    scalar2=CLAMP,
                    op0=mybir.AluOpType.max,
                    op1=mybir.AluOpType.min,
                )

            with (
                tc.tile_pool(name="ps", bufs=6, space="PSUM") as psp,
                tc.tile_pool(name="psh", bufs=2, space="PSUM") as psh,
            ):
                for c in range(0, NCHUNK - 2, 2):
                    # tap-major over the chunk pair (both H-halves of depth
                    # slice d): consecutive matmuls share the same weight tap
                    d = c // 2
                    if d + 1 < D:
                        # stage the next depth slice one chunk-pair ahead
                        load_slice(d + 1)
                    valid = _valid_taps(d)
                    ocp = op.tile([COUT, 2, 512], io_dt, name=f"oc_{c}", tag="oc")
                    ps_a = psp.tile([COUT, 512], dt.float32, name=f"ps_{c}", tag="ps")
                    ps_b = psp.tile(
                        [COUT, 512], dt.float32, name=f"ps_{c + 1}", tag="ps"
                    )
                    for t in valid:
                        kd, kh, kw = t // 9, (t // 3) % 3, t % 3
                        for hi, ps in ((0, ps_a), (1, ps_b)):
                            h0 = hi * 16
                            rhs = xpad[
                                :, d + kd - 1, h0 + kh : h0 + kh + 16, kw : kw + 32
                            ]
                            nc.tensor.matmul(
                                ps[:],
                                w_sb[:, t, :],
                                rhs,
                                start=(t == valid[0]),
                                stop=(t == valid[-1]),
                            )
                    epilogue(ps_a[:], ocp[:, 0, :], 512, c)
                    epilogue(ps_b[:], ocp[:, 1, :], 512, c + 1)
                    nc.scalar.dma_start(y_d[:, c : c + 2, :], ocp[:])

                for c in (NCHUNK - 2, NCHUNK - 1):
                    d, h0 = c // 2, (c % 2) * 16
                    valid = _valid_taps(d)
                    if c == NCHUNK - 2:
                        ps = psp.tile([COUT, 512], dt.float32, name=f"ps_{c}", tag="ps")
                        for t in valid:
                            kd, kh, kw = t // 9, (t // 3) % 3, t % 3
                            rhs = xpad[
                                :, d + kd - 1, h0 + kh : h0 + kh + 16, kw : kw + 32
                            ]
                            nc.tensor.matmul(
                                ps[:],
                                w_sb[:, t, :],
                                rhs,
                                start=(t == valid[0]),
                                stop=(t == valid[-1]),
                            )
                        oc = op.tile([COUT, 512], io_dt, name=f"oc_{c}", tag="oc")
                        epilogue(ps[:], oc[:], 512, c)
                        nc.scalar.dma_start(y_d[:, c, :], oc[:])
                    else:
                        # split the last chunk so its epilogue overlaps matmuls
                        for hi in range(2):
                            hh = h0 + 8 * hi
                            ph = psh.tile(
                                [COUT, 256], dt.float32, name=f"ps_{c}_{hi}", tag="psh"
                            )
                            for t in valid:
                                kd, kh, kw = t // 9, (t // 3) % 3, t % 3
                                rhs = xpad[
                                    :, d + kd - 1, hh + kh : hh + kh + 8, kw : kw + 32
                                ]
                                nc.tensor.matmul(
                                    ph[:],
                                    w_sb[:, t, :],
                                    rhs,
                                    start=(t == valid[0]),
                                    stop=(t == valid[-1]),
                                )
                            oc = op.tile(
                                [COUT, 256], io_dt, name=f"oc_{c}_{hi}", tag="oc"
                            )
                            epilogue(ph[:], oc[:], 256, c, half=f"_{hi}")
                            nc.scalar.dma_start(
                                y_d[:, c, 256 * hi : 256 * (hi + 1)], oc[:]
                            )
    nc.compile()
    return nc


def _get_nc():
    if "nc" not in _CACHED:
        _CACHED["nc"] = _build_nc()
    return _CACHED["nc"]


def kernel(x: np.ndarray, weight: np.ndarray, bias: np.ndarray) -> np.ndarray:
    global LAST_RESULTS
    import ml_dtypes

    io = ml_dtypes.bfloat16

    x = np.asarray(x)
    weight = np.asarray(weight, dtype=np.float32)
    bias = np.asarray(bias, dtype=np.float32)

    # demodulation coefficients (styles == ones) folded into the weights
    dcoef = 1.0 / np.sqrt(
        np.sum(np.square(weight.astype(np.float64)), axis=(1, 2, 3, 4)) + EPS
    )
    w_fold = weight * dcoef[:, None, None, None, None].astype(np.float32)
    # [cout, cin, kd, kh, kw] -> [cin, (kd kh kw), cout]
    w_prep = np.ascontiguousarray(
        w_fold.transpose(1, 2, 3, 4, 0).reshape(CIN, NTAPS, COUT).astype(io)
    )
    # epilogue computes max(alpha*u, u) with u = sqrt2*psum + sqrt2*bias
    b_prep = np.ascontiguousarray((S1 * bias).reshape(COUT, 1).astype(np.float32))

    in_maps = [
        {
            "x": np.ascontiguousarray(x[i].astype(io)),
            "w": w_prep,
            "bias": b_prep,
        }
        for i in range(B)
    ]

    nc = _get_nc()
    trace = bool(int(os.environ.get("CONV_TRACE", "0")))
    res = run_bass_kernel_spmd(
        nc,
        in_maps,
        core_ids=list(range(B)),
        trace=trace,
    )
    LAST_RESULTS = res
    out = np.stack(
        [r["y"].astype(np.float32).reshape(COUT, D, H, W) for r in res.results],
        axis=0,
    )
    return out


# revision 11
# speedup vs baseline: 1.0046x; 1.0046x over previous
"""Trainium2 Bass kernel for nn_Conv3DNorm (modulated conv3d + demod + lrelu + clamp).

Reference math (styles == ones):
    dcoef[cout] = rsqrt(sum_{cin,kd,kh,kw} weight^2 + 1e-8)
    y = conv3d(x, weight * dcoef, pad=1)            # per-sample, stride 1
    y = leaky_relu(y + bias, 0.2) * sqrt(2)
    y = clip(y, -256, 256)

Sharding: data-parallel over batch. Core i processes sample i (B=8 == n_cores).

Design (v3):
  - dcoef folded into the weights on the host (it only depends on `weight`),
    so the device runs a plain conv + lrelu + clamp.
  - conv = 27 accumulated bf16 matmuls per 512-position output chunk over a
    zero-padded (H,W) input volume in SBUF. bf16 halves the PE weight-load
    (LDWEIGHTS) time vs f32r so it hides completely under the 512-row matmul
    stream (issue interval ~219 ns vs ~255 ns for f32r).
  - input path: each x depth-slice DMAs contiguously (128 x 2KB descriptors)
    into a staging ring on the sync HWDGE queue; a DVE copy scatters it into
    the padded layout right after a cheap contiguous full-plane memset (the
    old strided column memsets cost ~4.4 us each on DVE and delayed the
    first matmul).
  - weights are split into 3 tap-range DMAs with the first-needed taps
    (9..17, the kd=1 block used by the d=0 chunks) landing before the x
    slices on the sync queue; the rest follow. bias rides the Act queue.
  - y goes out over the Activation-engine HWDGE queue; the SWDGE path is
    completely unused, avoiding its ~6.5 us drain at teardown.
  - epilogue per chunk (exact lrelu, no act-table dependency):
        u  = psum*sqrt2 + sqrt2*bias      (Identity activation, Act engine)
        o  = max(alpha*u, u)              (DVE scalar_tensor_tensor)
        oc = clip(o, +-256) -> bf16       (DVE tensor_scalar)
    using lrelu(z) = max(z, alpha*z) for 0 < alpha < 1.
  - last chunk split into two 256-wide halves so its epilogue/DMA overlap
    the final matmuls.
"""

import os
import sys

for _p in (
    "/root/.axon_site",
    "/root/.axon_site/_ro/trn_rl_repo",
    "/root/.axon_site/_ro/pypackages",
):
    if os.path.isdir(_p) and _p not in sys.path:
        sys.path.insert(0, _p)

import numpy as np

import concourse.bass as bass  # noqa: F401
import concourse.mybir as mybir
import concourse.tile as tile
from concourse import bacc
from concourse.bass_utils import run_bass_kernel_spmd

# Problem constants (hardcoded per contract).
B = 8
CIN = 128
COUT = 128
D = H = W = 32
K = 3
NTAPS = K * K * K  # 27
HP = H + 2  # 34
WP = W + 2  # 34
NCHUNK = 64  # output chunks of 512 spatial positions: (d, half-of-H)
EPS = 1e-8
S1 = float(np.sqrt(2.0))  # ACT_GAIN * GAIN
CLAMP = 256.0
ALPHA = 0.2

LAST_RESULTS = None  # BassKernelResults of the most recent run (for test.py)

_CACHED = {}


def _valid_taps(d):
    return [t for t in range(NTAPS) if 0 <= d + t // 9 - 1 < D]


def _build_nc():
    dt = mybir.dt
    io_dt = dt.bfloat16

    nc = bacc.Bacc("TRN2")
    x_d = nc.dram_tensor("x", [CIN, D, H, W], io_dt, kind="ExternalInput")
    w_d = nc.dram_tensor("w", [CIN, NTAPS, COUT], io_dt, kind="ExternalInput")
    b_d = nc.dram_tensor("bias", [COUT, 1], dt.float32, kind="ExternalInput")
    y_d = nc.dram_tensor("y", [COUT, NCHUNK, 512], io_dt, kind="ExternalOutput")

    with tile.TileContext(nc) as tc:
        with (
            tc.tile_pool(name="big", bufs=1) as big,
            tc.tile_pool(name="small", bufs=1) as small,
            tc.tile_pool(name="stg", bufs=4) as stp,
            tc.tile_pool(name="epiv", bufs=4) as vp,
            tc.tile_pool(name="epio", bufs=4) as op,
        ):
            w_sb = big.tile([CIN, NTAPS, COUT], io_dt)
            bias_sb = small.tile([COUT, 1], dt.float32)
            xpad = big.tile([CIN, D, HP, WP], io_dt)

            def load_slice(d):
                # contiguous plane memset (halo zeroing) + contiguous DMA
                # (128 x 2KB descriptors) + DVE scatter into padded layout
                nc.vector.memset(xpad[:, d, :, :], 0.0)
                st = stp.tile([CIN, H, W], io_dt, name=f"st_{d}", tag="st")
                nc.sync.dma_start(st[:], x_d[:, d, :, :])
                nc.vector.tensor_scalar_add(
                    xpad[:, d, 1 : HP - 1, 1 : WP - 1], st[:], 0.0
                )

            # sync-queue order tuned so the first matmul's inputs land first:
            # slice 0, tap 9's weights (the first LDWEIGHTS), slice 1, then
            # the remaining weight taps in consumption order
            load_slice(0)
            nc.sync.dma_start(w_sb[:, 9:10, :], w_d[:, 9:10, :])
            nc.sync.dma_start(w_sb[:, 10:18, :], w_d[:, 10:18, :])
            load_slice(1)
            nc.sync.dma_start(w_sb[:, 18:27, :], w_d[:, 18:27, :])
            nc.sync.dma_start(w_sb[:, 0:9, :], w_d[:, 0:9, :])
            nc.scalar.dma_start(bias_sb[:], b_d[:])

            # garbage operand for the PE warm-up matmuls, zeroed early on the
            # otherwise-idle GpSimd engine
            warm_sb = small.tile([CIN, 256], io_dt, name="warm")
            nc.gpsimd.memset(warm_sb[:], 0.0)

            def epilogue(ps_ap, oc_ap, width, c, half=""):
                # u = sqrt2*psum + sqrt2*bias; o = max(alpha*u, u) = sqrt2*lrelu
                u = vp.tile([COUT, width], dt.float32)
                nc.scalar.activation(
                    u[:],
                    ps_ap,
                    mybir.ActivationFunctionType.Identity,
                    bias=bias_sb[:],
                    scale=S1,
                )
                o = vp.tile([COUT, width], dt.float32, name=f"o_{c}{half}", tag="o")
                nc.vector.scalar_tensor_tensor(
                    out=o[:],
                    in0=u[:],
                    scalar=ALPHA,
                    in1=u[:],
                    op0=mybir.AluOpType.mult,
                    op1=mybir.AluOpType.max,
                )
                nc.vector.tensor_scalar(
                    out=oc_ap,
                    in0=o[:],
                    scalar1=-CLAMP,
                    scalar2=CLAMP,
                    op0=mybir.AluOpType.max,
                    op1=mybir.AluOpType.min,
                )

            with (
                tc.tile_pool(name="ps", bufs=6, space="PSUM") as psp,
                tc.tile_pool(name="psh", bufs=2, space="PSUM") as psh,
            ):
                # PE warm-up: garbage matmuls fill the ~7->10.6us window
                # before the first input slice is ready, so the p-state ramp
                # (1.2 GHz until ~3us of continuous busy) is paid on garbage
                # instead of the real matmul stream
                ps_warm = psh.tile([COUT, 256], dt.float32, name="ps_warm", tag="psh")
                for _ in range(14):
                    nc.tensor.matmul(
                        ps_warm[:],
                        warm_sb[:, 0:128],
                        warm_sb[:],
                        start=True,
                        stop=True,
                        skip_group_check=True,
                    )

                ocp = None
                for c in range(NCHUNK - 2):
                    d, h0 = c // 2, (c % 2) * 16
                    if c % 2 == 0 and d + 1 < D:
                        # stage the next depth slice one chunk-pair ahead
                        load_slice(d + 1)
                    valid = _valid_taps(d)
                    if c % 2 == 0:
                        # y DMAs are paired (one DMA per two chunks)
                        ocp = op.tile([COUT, 2, 512], io_dt, name=f"oc_{c}", tag="oc")
                    ps = psp.tile([COUT, 512], dt.float32, name=f"ps_{c}", tag="ps")
                    for t in valid:
                        kd, kh, kw = t // 9, (t // 3) % 3, t % 3
                        rhs = xpad[
                            :, d + kd - 1, h0 + kh : h0 + kh + 16, kw : kw + 32
                        ]
                        nc.tensor.matmul(
                            ps[:],
                            w_sb[:, t, :],
                            rhs,
                            start=(t == valid[0]),
                            stop=(t == valid[-1]),
                        )
                    epilogue(ps[:], ocp[:, c % 2, :], 512, c)
                    if c % 2 == 1:
                        nc.scalar.dma_start(y_d[:, c - 1 : c + 1, :], ocp[:])

                for c in (NCHUNK - 2, NCHUNK - 1):
                    d, h0 = c // 2, (c % 2) * 16
                    valid = _valid_taps(d)
                    if c == NCHUNK - 2:
                        ps = psp.tile([COUT, 512], dt.float32, name=f"ps_{c}", tag="ps")
                        for t in valid:
                            kd, kh, kw = t // 9, (t // 3) % 3, t % 3
                            rhs = xpad[
                                :, d + kd - 1, h0 + kh : h0 + kh + 16, kw : kw + 32
                            ]
                            nc.tensor.matmul(
                                ps[:],
                                w_sb[:, t, :],
                                rhs,
                                start=(t == valid[0]),
                                stop=(t == valid[-1]),
                            )
                        oc = op.tile([COUT, 512], io_dt, name=f"oc_{c}", tag="oc")
                        epilogue(ps[:], oc[:], 512, c)
                        nc.scalar.dma_start(y_d[:, c, :], oc[:])
                    else:
                        # split the last chunk so its epilogue overlaps matmuls
                        for hi in range(2):
                            hh = h0 + 8 * hi
                            ph = psh.tile(
                                [COUT, 256], dt.float32, name=f"ps_{c}_{hi}", tag="psh"
                            )
                            for t in valid:
                                kd, kh, kw = t // 9, (t // 3) % 3, t % 3
                                rhs = xpad[
                                    :, d + kd - 1, hh + kh : hh + kh + 8, kw : kw + 32
                                ]
                                nc.tensor.matmul(
                                    ph[:],
                                    w_sb[:, t, :],
                                    rhs,
                                    start=(t == valid[0]),
                                    stop=(t == valid[-1]),
                                )
                            oc = op.tile(
                                [COUT, 256], io_dt, name=f"oc_{c}_{hi}", tag="oc"
                            )
                            epilogue(ph[:], oc[:], 256, c, half=f"_{hi}")
                            nc.scalar.dma_start(
                                y_d[:, c, 256 * hi : 256 * (hi + 1)], oc[:]
                            )
    nc.compile()
    return nc


def _get_nc():
    if "nc" not in _CACHED:
        _CACHED["nc"] = _build_nc()
    return _CACHED["nc"]


def kernel(x: np.ndarray, weight: np.ndarray, bias: np.ndarray) -> np.ndarray:
    global LAST_RESULTS
    import ml_dtypes

    io = ml_dtypes.bfloat16

    x = np.asarray(x)
    weight = np.asarray(weight, dtype=np.float32)
    bias = np.asarray(bias, dtype=np.float32)

    # demodulation coefficients (styles == ones) folded into the weights
    dcoef = 1.0 / np.sqrt(
        np.sum(np.square(weight.astype(np.float64)), axis=(1, 2, 3, 4)) + EPS
    )
    w_fold = weight * dcoef[:, None, None, None, None].astype(np.float32)
    # [cout, cin, kd, kh, kw] -> [cin, (kd kh kw), cout]
    w_prep = np.ascontiguousarray(
        w_fold.transpose(1, 2, 3, 4, 0).reshape(CIN, NTAPS, COUT).astype(io)
    )
    # epilogue computes max(alpha*u, u) with u = sqrt2*psum + sqrt2*bias
    b_prep = np.ascontiguousarray((S1 * bias).reshape(COUT, 1).astype(np.float32))

    in_maps = [
        {
            "x": np.ascontiguousarray(x[i].astype(io)),
            "w": w_prep,
            "bias": b_prep,
        }
        for i in range(B)
    ]

    nc = _get_nc()
    trace = bool(int(os.environ.get("CONV_TRACE", "0")))
    res = run_bass_kernel_spmd(
        nc,
        in_maps,
        core_ids=list(range(B)),
        trace=trace,
    )
    LAST_RESULTS = res
    out = np.stack(
        [r["y"].astype(np.float32).reshape(COUT, D, H, W) for r in res.results],
        axis=0,
    )
    return out
